# revision 22
# baseline (speedup 1.0000x reference)
"""Trainium2 Bass kernel for nn_FAIIAHead (focal-attention prototype head).

Reference computation (per sample, D_IN=512, D_ATT=32, N_PROTO=4):
    q       = x @ Wq.T + bq
    scores  = q @ proto_keys.T + proto_importance
    scores *= (1 + focal_alpha * (u + 1e-8)^2 * focal_temp),  u = 1 - 2|mp - 0.5|
    weights = softmax(scores * D_ATT^-0.5)
    out     = LN(weights @ proto_values @ Wo.T + bo) * ln_gamma + ln_beta
    returns (out, weights)

q is never an output, so the two projection chains fold on the host:
    Wk' = proto_keys @ Wq            [4, 512]   scores = x @ Wk'.T + c0
    c0  = bq @ proto_keys.T + imp    [4]        (uniform c0 cancels in softmax)
    Wv' = proto_values @ Wo.T        [4, 32]    pre_ln = weights @ Wv' + bo

This cuts tensor-engine work 8x and makes the kernel HBM-bound on streaming x.

Per-core dataflow (pure data parallel over batch, 8 cores):
  - batch mapped chunk-wise: chunk = 4096 rows, row b = chunk*4096 + 32*p + s
    (p = SBUF partition, s = slot 0..31) so every DMA is >=512B contiguous
    per partition on both loads and stores.
  - x loaded with f32->f16 cast in the DMA (SWDGE), then each [128,128]
    block is transposed on the PE via a plain matmul against identity
    (exact for f16 values, keeps the PE clock-gate warm).
  - scores: lhsT = xT block (self-loading f16 weights, FWL), rhs = Wk'.T
    chunk [128,4], accumulated over 4 K-chunks straight into natural
    layout [128 batch, 4 proto] in PSUM. No score transposes needed.
  - softmax on DVE/ACT in natural layout (logits are bounded ~|0.5|, so no
    max-subtraction is needed; exp/ln share one ACT table set).
  - out-projection: weights [128,128] transposed once per chunk on the PE,
    then one K=128 matmul against a host-built block-diagonal [128,1024]
    expansion of Wv' (f32) produces all 32 slots' pre-LN outputs at once.
  - LayerNorm on DVE in f32; rstd = exp(-0.5*ln(var+eps)) to stay inside
    the exp/ln ACT table set (ACT sqrt has a loose precision budget).
"""

import numpy as np

_BASS_REPO = "/opt/trn_rl_repo"

B_TOTAL = 262144
D_IN = 512
D_ATT = 32
N_PROTO = 4
GAMMA = 2.0
SCALE = D_ATT ** -0.5
LN_EPS = 1e-5
N_CORES = 8

S = 32                # batch slots per partition per chunk
CHUNK = 128 * S       # 4096 rows per chunk
KC = D_IN // 128      # 4 contraction chunks

_NC_CACHE = {}


def _import_bass():
    import sys
    if _BASS_REPO not in sys.path:
        sys.path.insert(0, _BASS_REPO)


def _expand(ap, pos, count, bass):
    """Insert a broadcast (step 0) free dim of `count` at free-dim index
    `pos` (0 = first free dim) into an SBUF/PSUM access pattern."""
    dims = [list(d) for d in ap.ap]
    dims.insert(1 + pos, [0, count])
    return bass.AP(tensor=ap.tensor, offset=ap.offset, ap=dims)


def _build_nc(b_core, g_a, g_c, add_c0, use_bo, use_gamma, use_beta,
              copy_split=9, copy_mod=16):
    _import_bass()
    import concourse.bass as bass
    import concourse.bacc as bacc
    import concourse.tile as tile
    from concourse import mybir

    f32 = mybir.dt.float32
    f16 = mybir.dt.float16
    AF = mybir.ActivationFunctionType
    ALU = mybir.AluOpType
    AX = mybir.AxisListType

    # chunk schedule in 128-row slot units: full 32-slot chunks with a
    # tapered tail so the final compute drain is short.
    total_slots = b_core // 128
    assert b_core % 128 == 0
    sizes = []
    rem = total_slots
    while rem >= S:
        sizes.append(S)
        rem -= S
    while rem >= 4:
        sizes.append(4)
        rem -= 4
    assert rem == 0, f"b_core={b_core} not a multiple of 512"

    nc = bacc.Bacc("TRN2", target_bir_lowering=False, debug=False,
                   num_devices=N_CORES)

    x_d = nc.dram_tensor("x", [b_core, D_IN], f32, kind="ExternalInput").ap()
    mp_d = nc.dram_tensor("mp", [b_core], f32, kind="ExternalInput").ap()
    wk_d = nc.dram_tensor("wk", [D_IN, N_PROTO], f16, kind="ExternalInput").ap()
    wvb_d = nc.dram_tensor("wvb", [128, S * D_ATT], f32, kind="ExternalInput").ap()
    id16_d = nc.dram_tensor("id16", [128, 128], f16, kind="ExternalInput").ap()
    id32_d = nc.dram_tensor("id32", [128, 128], f32, kind="ExternalInput").ap()
    if add_c0:
        c0_d = nc.dram_tensor("c0", [N_PROTO], f32, kind="ExternalInput").ap()
    if use_bo:
        bo_d = nc.dram_tensor("bo", [D_ATT], f32, kind="ExternalInput").ap()
    if use_gamma:
        gam_d = nc.dram_tensor("gam", [D_ATT], f32, kind="ExternalInput").ap()
    if use_beta:
        bet_d = nc.dram_tensor("bet", [D_ATT], f32, kind="ExternalInput").ap()
    out_d = nc.dram_tensor("out", [b_core, D_ATT], f32, kind="ExternalOutput").ap()
    w_d = nc.dram_tensor("wout", [b_core, N_PROTO], f32, kind="ExternalOutput").ap()

    def bcast_load(dram_ap, n):
        src = bass.AP(tensor=dram_ap.tensor, offset=dram_ap.offset,
                      ap=[[0, 128], [1, n]])
        return src

    with tile.TileContext(nc) as tc:
        with (
            tc.tile_pool(name="singles", bufs=1) as singles,
            tc.tile_pool(name="xin", bufs=6) as xin_pool,
            tc.tile_pool(name="xtsb", bufs=12) as xtsb_pool,
            tc.tile_pool(name="small", bufs=4) as small_pool,
            tc.tile_pool(name="smx", bufs=3) as smx_pool,
            tc.tile_pool(name="wout", bufs=3) as wout_pool,
            tc.tile_pool(name="ln", bufs=3) as ln_pool,
            tc.tile_pool(name="obuf", bufs=3) as out_pool,
            tc.tile_pool(name="xtps", bufs=3, space="PSUM") as xtps_pool,
            tc.tile_pool(name="scps", bufs=2, space="PSUM") as scps_pool,
            tc.tile_pool(name="wtps", bufs=1, space="PSUM") as wtps_pool,
            tc.tile_pool(name="ops", bufs=1, space="PSUM") as ops_pool,
        ):
            # ---- one-time parameter loads ----
            wk_sb = singles.tile([128, KC, N_PROTO], f16)
            nc.sync.dma_start(
                out=wk_sb, in_=wk_d.rearrange("(a p) q -> p a q", p=128))
            wvb_sb = singles.tile([128, S * D_ATT], f32)
            nc.sync.dma_start(out=wvb_sb, in_=wvb_d)
            id16 = singles.tile([128, 128], f16)
            nc.sync.dma_start(out=id16, in_=id16_d)
            id32 = singles.tile([128, 128], f32)
            nc.sync.dma_start(out=id32, in_=id32_d)
            eps_t = singles.tile([128, 1], f32)
            nc.vector.memset(eps_t, LN_EPS)
            if add_c0:
                c0b = singles.tile([128, N_PROTO], f32)
                nc.sync.dma_start(out=c0b, in_=bcast_load(c0_d, N_PROTO))
            if use_bo:
                bob = singles.tile([128, D_ATT], f32)
                nc.sync.dma_start(out=bob, in_=bcast_load(bo_d, D_ATT))
            if use_gamma:
                gamb = singles.tile([128, D_ATT], f32)
                nc.sync.dma_start(out=gamb, in_=bcast_load(gam_d, D_ATT))
            if use_beta:
                betb = singles.tile([128, D_ATT], f32)
                nc.sync.dma_start(out=betb, in_=bcast_load(bet_d, D_ATT))

            copy_idx = 0
            b0 = 0
            for sc in sizes:
                rows = 128 * sc
                x_c = x_d[b0:b0 + rows].rearrange("(p s) k -> p s k", s=sc)
                mp_c = mp_d[b0:b0 + rows].rearrange("(p s) -> p s", s=sc)
                w_c = w_d[b0:b0 + rows].rearrange("(p s) q -> p s q", s=sc)
                out_c = out_d[b0:b0 + rows].rearrange("(p s) d -> p s d", s=sc)
                b0 += rows
                # ---- focal modulation factor g = SCALE*(1+a*t*u^2) ----
                mp_t = small_pool.tile([128, sc], f32, tag="mp")
                nc.sync.dma_start(out=mp_t, in_=mp_c)
                t_t = small_pool.tile([128, sc], f32, tag="gt")
                nc.vector.tensor_scalar(out=t_t, in0=mp_t, scalar1=2.0,
                                        scalar2=-1.0, op0=ALU.mult,
                                        op1=ALU.add)
                tn_t = small_pool.tile([128, sc], f32, tag="gtn")
                nc.vector.tensor_scalar_mul(out=tn_t, in0=t_t, scalar1=-1.0)
                a_t = small_pool.tile([128, sc], f32, tag="ga")
                nc.vector.tensor_max(out=a_t, in0=t_t, in1=tn_t)
                u_t = small_pool.tile([128, sc], f32, tag="gu")
                nc.vector.tensor_scalar(out=u_t, in0=a_t, scalar1=-1.0,
                                        scalar2=1.0 + 1e-8, op0=ALU.mult,
                                        op1=ALU.add)
                u2_t = small_pool.tile([128, sc], f32, tag="gu2")
                nc.vector.tensor_mul(out=u2_t, in0=u_t, in1=u_t)
                g_t = small_pool.tile([128, sc], f32, tag="gg")
                nc.vector.tensor_scalar(out=g_t, in0=u2_t, scalar1=g_a,
                                        scalar2=g_c, op0=ALU.mult, op1=ALU.add)

                # ---- x load (8-slot DMAs) + PE transposes + scores ----
                scores_ps = scps_pool.tile([128, sc, N_PROTO], f32)
                x_ts = []
                for lg in range(0, sc, 4):
                    x_t = xin_pool.tile([128, 4, D_IN], f16, tag="x")
                    nc.gpsimd.dma_start(out=x_t, in_=x_c[:, lg:lg + 4, :])
                    x_ts.append((lg, 4, x_t))
                for lg, w8, x_t in x_ts:
                    for sg4 in range(0, w8, 4):
                        xt_sbs = []
                        for kc in range(KC):
                            xt_ps = xtps_pool.tile([128, 4, 128], f32)
                            for jj in range(4):
                                # xT block = x_block.T @ I  (f16 matmul)
                                nc.tensor.matmul(
                                    out=xt_ps[:, jj, :],
                                    lhsT=x_t[:, sg4 + jj, 128 * kc:128 * (kc + 1)],
                                    rhs=id16,
                                    start=True, stop=True)
                            xt_sb = xtsb_pool.tile([128, 4, 128], f16, tag="xt")
                            if (copy_idx % copy_mod) < copy_split:
                                nc.scalar.copy(out=xt_sb[:], in_=xt_ps[:])
                            else:
                                nc.vector.tensor_copy(out=xt_sb[:], in_=xt_ps[:])
                            copy_idx += 1
                            xt_sbs.append(xt_sb)
                        for jj in range(4):
                            j = lg + sg4 + jj
                            for kc in range(KC):
                                nc.tensor.matmul(
                                    out=scores_ps[:, j, :],
                                    lhsT=xt_sbs[kc][:, jj, :],
                                    rhs=wk_sb[:, kc, :],
                                    start=(kc == 0), stop=(kc == KC - 1))

                # ---- softmax over prototypes (natural layout) ----
                l_t = smx_pool.tile([128, sc, N_PROTO], f32, tag="logit")
                if add_c0:
                    nc.vector.tensor_add(out=l_t, in0=scores_ps,
                                         in1=_expand(c0b[:], 0, sc, bass))
                    nc.vector.tensor_mul(out=l_t, in0=l_t,
                                         in1=_expand(g_t[:], 1, N_PROTO, bass))
                else:
                    nc.vector.tensor_mul(out=l_t, in0=scores_ps,
                                         in1=_expand(g_t[:], 1, N_PROTO, bass))
                e_t = smx_pool.tile([128, sc, N_PROTO], f32, tag="esc")
                nc.scalar.activation(out=e_t, in_=l_t, func=AF.Exp)
                z_t = small_pool.tile([128, sc], f32, tag="zsum")
                nc.vector.tensor_reduce(out=z_t, in_=e_t, axis=AX.X, op=ALU.add)
                rz_t = small_pool.tile([128, sc], f32, tag="rz")
                nc.vector.reciprocal(out=rz_t, in_=z_t)
                w_t = wout_pool.tile([128, sc, N_PROTO], f32, tag="w")
                nc.vector.tensor_mul(out=w_t, in0=e_t,
                                     in1=_expand(rz_t[:], 1, N_PROTO, bass))
                nc.sync.dma_start(out=w_c, in_=w_t)

                # ---- weights.T then block-diag Wv' matmul -> pre-LN ----
                kq = N_PROTO * sc
                wt_ps = wtps_pool.tile([128, 128], f32)
                w_flat = w_t[:].rearrange("p s q -> p (s q)")
                nc.tensor.matmul(out=wt_ps[:kq, :], lhsT=w_flat, rhs=id32,
                                 start=True, stop=True)
                wt_sb = smx_pool.tile([128, 128], f32, tag="wt")
                nc.scalar.copy(out=wt_sb[:kq, :], in_=wt_ps[:kq, :])


                outp = ops_pool.tile([128, sc, D_ATT], f32)
                for h0 in range(0, sc * D_ATT, 512):
                    hw = min(512, sc * D_ATT - h0)
                    nc.tensor.matmul(
                        out=outp[:, h0 // D_ATT:(h0 + hw) // D_ATT, :],
                        lhsT=wt_sb[:kq, :],
                        rhs=wvb_sb[:kq, h0:h0 + hw],
                        start=True, stop=True)
                ln_src = outp[:]
                if use_bo:
                    y_t = ln_pool.tile([128, sc, D_ATT], f32, tag="y")
                    nc.vector.tensor_add(out=y_t, in0=outp,
                                         in1=_expand(bob[:], 0, sc, bass))
                    ln_src = y_t[:]
                s1_t = small_pool.tile([128, sc], f32, tag="lnsum")
                nc.vector.tensor_reduce(out=s1_t, in_=ln_src, axis=AX.X,
                                        op=ALU.add)
                m_t = small_pool.tile([128, sc], f32, tag="lnmean")
                nc.vector.tensor_scalar_mul(out=m_t, in0=s1_t,
                                            scalar1=1.0 / D_ATT)
                c_t = ln_pool.tile([128, sc, D_ATT], f32, tag="lncen")
                nc.vector.tensor_sub(out=c_t, in0=ln_src,
                                     in1=_expand(m_t[:], 1, D_ATT, bass))
                q_t = ln_pool.tile([128, sc, D_ATT], f32, tag="lnsq")
                nc.scalar.square(out=q_t[:], in_=c_t[:])
                v_t = small_pool.tile([128, sc], f32, tag="lnvar")
                nc.vector.tensor_reduce(out=v_t, in_=q_t, axis=AX.X,
                                        op=ALU.add)
                lnv_t = small_pool.tile([128, sc], f32, tag="lnlog")
                nc.scalar.activation(out=lnv_t, in_=v_t, func=AF.Ln,
                                     scale=1.0 / D_ATT, bias=eps_t[:])
                r_t = small_pool.tile([128, sc], f32, tag="lnrstd")
                nc.scalar.activation(out=r_t, in_=lnv_t, func=AF.Exp,
                                     scale=-0.5)
                o_t = out_pool.tile([128, sc, D_ATT], f32, tag="o")
                nc.vector.tensor_mul(out=o_t, in0=c_t,
                                     in1=_expand(r_t[:], 1, D_ATT, bass))
                if use_gamma:
                    nc.vector.tensor_mul(out=o_t, in0=o_t,
                                         in1=_expand(gamb[:], 0, sc, bass))
                if use_beta:
                    nc.vector.tensor_add(out=o_t, in0=o_t,
                                         in1=_expand(betb[:], 0, sc, bass))
                nc.sync.dma_start(out=out_c, in_=o_t)

    nc.compile()
    return nc


def _prepare(inputs):
    """Fold parameters on the host, build (or reuse) the Bass program and
    the 8 per-core input maps."""
    x = np.ascontiguousarray(np.asarray(inputs["x"], dtype=np.float32))
    mp = np.ascontiguousarray(np.asarray(inputs["minority_prob"], np.float32))
    Wq = np.asarray(inputs["Wq"], np.float32)
    bq = np.asarray(inputs["bq"], np.float32)
    pk = np.asarray(inputs["proto_keys"], np.float32)
    pv = np.asarray(inputs["proto_values"], np.float32)
    imp = np.asarray(inputs["proto_importance"], np.float32)
    alpha = float(np.asarray(inputs["focal_alpha"], np.float32).reshape(-1)[0])
    temp = float(np.asarray(inputs["focal_temp"], np.float32).reshape(-1)[0])
    Wo = np.asarray(inputs["Wo"], np.float32)
    bo = np.asarray(inputs["bo"], np.float32)
    gam = np.asarray(inputs["ln_gamma"], np.float32)
    bet = np.asarray(inputs["ln_beta"], np.float32)

    b_total = x.shape[0]
    assert b_total % N_CORES == 0
    b_core = b_total // N_CORES

    Wk = pk @ Wq                                     # [4, 512]
    wk16 = np.ascontiguousarray(Wk.T).astype(np.float16)
    c0 = bq @ pk.T + imp                             # [4]
    add_c0 = not bool(np.all(np.abs(c0 - c0[0]) < 1e-30))
    Wv = pv @ Wo.T                                   # [4, 32]
    wvb = np.zeros((128, S * D_ATT), np.float32)
    for j in range(S):
        wvb[N_PROTO * j:N_PROTO * (j + 1), D_ATT * j:D_ATT * (j + 1)] = Wv
    g_a = SCALE * alpha * temp
    g_c = SCALE
    use_bo = bool(np.any(bo != 0.0))
    use_gamma = bool(np.any(gam != 1.0))
    use_beta = bool(np.any(bet != 0.0))

    key = (b_core, round(g_a, 12), round(g_c, 12), add_c0, use_bo,
           use_gamma, use_beta)
    if key not in _NC_CACHE:
        _NC_CACHE[key] = _build_nc(b_core, g_a, g_c, add_c0, use_bo,
                                   use_gamma, use_beta)
    nc = _NC_CACHE[key]

    id16 = np.eye(128, dtype=np.float16)
    id32 = np.eye(128, dtype=np.float32)
    in_maps = []
    for i in range(N_CORES):
        m = {
            "x": x[i * b_core:(i + 1) * b_core],
            "mp": mp[i * b_core:(i + 1) * b_core],
            "wk": wk16,
            "wvb": wvb,
            "id16": id16,
            "id32": id32,
        }
        if add_c0:
            m["c0"] = np.ascontiguousarray(c0)
        if use_bo:
            m["bo"] = np.ascontiguousarray(bo)
        if use_gamma:
            m["gam"] = np.ascontiguousarray(gam)
        if use_beta:
            m["bet"] = np.ascontiguousarray(bet)
        in_maps.append(m)
    return nc, in_maps


def _run(inputs, trace=False, **kwargs):
    _import_bass()
    from concourse.bass_utils import run_bass_kernel_spmd
    nc, in_maps = _prepare(inputs)
    res = run_bass_kernel_spmd(nc, in_maps, core_ids=list(range(N_CORES)),
                               trace=trace, **kwargs)
    out = np.concatenate([r["out"] for r in res.results], axis=0)
    wout = np.concatenate([r["wout"] for r in res.results], axis=0)
    return (out, wout), res


def kernel(**inputs):
    (out, wout), _ = _run(inputs, trace=False)
    return out, wout


# revision 23
# speedup vs baseline: 1.0560x; 1.0560x over previous
"""Trainium2 Bass kernel for nn_FAIIAHead (focal-attention prototype head).

Reference computation (per sample, D_IN=512, D_ATT=32, N_PROTO=4):
    q       = x @ Wq.T + bq
    scores  = q @ proto_keys.T + proto_importance
    scores *= (1 + focal_alpha * (u + 1e-8)^2 * focal_temp),  u = 1 - 2|mp - 0.5|
    weights = softmax(scores * D_ATT^-0.5)
    out     = LN(weights @ proto_values @ Wo.T + bo) * ln_gamma + ln_beta
    returns (out, weights)

q is never an output, so the two projection chains fold on the host:
    Wk' = proto_keys @ Wq            [4, 512]   scores = x @ Wk'.T + c0
    c0  = bq @ proto_keys.T + imp    [4]        (uniform c0 cancels in softmax)
    Wv' = proto_values @ Wo.T        [4, 32]    pre_ln = weights @ Wv' + bo

This cuts tensor-engine work 8x and makes the kernel HBM-bound on streaming x.

Per-core dataflow (pure data parallel over batch, 8 cores):
  - batch mapped chunk-wise: chunk = 4096 rows, row b = chunk*4096 + 32*p + s
    (p = SBUF partition, s = slot 0..31) so every DMA is >=512B contiguous
    per partition on both loads and stores.
  - x loaded with f32->f16 cast in the DMA (SWDGE), then each [128,128]
    block is transposed on the PE via a plain matmul against identity
    (exact for f16 values, keeps the PE clock-gate warm).
  - scores: lhsT = xT block (self-loading f16 weights, FWL), rhs = Wk'.T
    chunk [128,4], accumulated over 4 K-chunks straight into natural
    layout [128 batch, 4 proto] in PSUM. No score transposes needed.
  - softmax on DVE/ACT in natural layout (logits are bounded ~|0.5|, so no
    max-subtraction is needed; exp/ln share one ACT table set).
  - out-projection: weights [128,128] transposed once per chunk on the PE,
    then one K=128 matmul against a host-built block-diagonal [128,1024]
    expansion of Wv' (f32) produces all 32 slots' pre-LN outputs at once.
  - LayerNorm on DVE in f32; rstd = exp(-0.5*ln(var+eps)) to stay inside
    the exp/ln ACT table set (ACT sqrt has a loose precision budget).
"""

import numpy as np

_BASS_REPO = "/opt/trn_rl_repo"

B_TOTAL = 262144
D_IN = 512
D_ATT = 32
N_PROTO = 4
GAMMA = 2.0
SCALE = D_ATT ** -0.5
LN_EPS = 1e-5
N_CORES = 8

S = 32                # batch slots per partition per chunk
CHUNK = 128 * S       # 4096 rows per chunk
KC = D_IN // 128      # 4 contraction chunks

_NC_CACHE = {}


def _import_bass():
    import sys
    if _BASS_REPO not in sys.path:
        sys.path.insert(0, _BASS_REPO)


def _expand(ap, pos, count, bass):
    """Insert a broadcast (step 0) free dim of `count` at free-dim index
    `pos` (0 = first free dim) into an SBUF/PSUM access pattern."""
    dims = [list(d) for d in ap.ap]
    dims.insert(1 + pos, [0, count])
    return bass.AP(tensor=ap.tensor, offset=ap.offset, ap=dims)


def _build_nc(b_core, g_a, g_c, add_c0, use_bo, use_gamma, use_beta,
              copy_split=14, copy_mod=16):
    _import_bass()
    import concourse.bass as bass
    import concourse.bacc as bacc
    import concourse.tile as tile
    from concourse import mybir

    f32 = mybir.dt.float32
    f16 = mybir.dt.float16
    AF = mybir.ActivationFunctionType
    ALU = mybir.AluOpType
    AX = mybir.AxisListType

    # chunk schedule in 128-row slot units: full 32-slot chunks with a
    # tapered tail so the final compute drain is short.
    total_slots = b_core // 128
    assert b_core % 128 == 0
    sizes = []
    rem = total_slots
    while rem >= S:
        sizes.append(S)
        rem -= S
    while rem >= 4:
        sizes.append(4)
        rem -= 4
    assert rem == 0, f"b_core={b_core} not a multiple of 512"

    nc = bacc.Bacc("TRN2", target_bir_lowering=False, debug=False,
                   num_devices=N_CORES)

    x_d = nc.dram_tensor("x", [b_core, D_IN], f32, kind="ExternalInput").ap()
    mp_d = nc.dram_tensor("mp", [b_core], f32, kind="ExternalInput").ap()
    wk_d = nc.dram_tensor("wk", [D_IN, N_PROTO], f16, kind="ExternalInput").ap()
    wvb_d = nc.dram_tensor("wvb", [128, S * D_ATT], f32, kind="ExternalInput").ap()
    id16_d = nc.dram_tensor("id16", [128, 128], f16, kind="ExternalInput").ap()
    id32_d = nc.dram_tensor("id32", [128, 128], f32, kind="ExternalInput").ap()
    if add_c0:
        c0_d = nc.dram_tensor("c0", [N_PROTO], f32, kind="ExternalInput").ap()
    if use_bo:
        bo_d = nc.dram_tensor("bo", [D_ATT], f32, kind="ExternalInput").ap()
    if use_gamma:
        gam_d = nc.dram_tensor("gam", [D_ATT], f32, kind="ExternalInput").ap()
    if use_beta:
        bet_d = nc.dram_tensor("bet", [D_ATT], f32, kind="ExternalInput").ap()
    out_d = nc.dram_tensor("out", [b_core, D_ATT], f32, kind="ExternalOutput").ap()
    w_d = nc.dram_tensor("wout", [b_core, N_PROTO], f32, kind="ExternalOutput").ap()

    def bcast_load(dram_ap, n):
        src = bass.AP(tensor=dram_ap.tensor, offset=dram_ap.offset,
                      ap=[[0, 128], [1, n]])
        return src

    with tile.TileContext(nc) as tc:
        with (
            tc.tile_pool(name="singles", bufs=1) as singles,
            tc.tile_pool(name="xin", bufs=6) as xin_pool,
            tc.tile_pool(name="xtsb", bufs=12) as xtsb_pool,
            tc.tile_pool(name="small", bufs=4) as small_pool,
            tc.tile_pool(name="smx", bufs=3) as smx_pool,
            tc.tile_pool(name="wout", bufs=3) as wout_pool,
            tc.tile_pool(name="ln", bufs=3) as ln_pool,
            tc.tile_pool(name="obuf", bufs=3) as out_pool,
            tc.tile_pool(name="xtps", bufs=3, space="PSUM") as xtps_pool,
            tc.tile_pool(name="scps", bufs=2, space="PSUM") as scps_pool,
            tc.tile_pool(name="wtps", bufs=1, space="PSUM") as wtps_pool,
            tc.tile_pool(name="ops", bufs=1, space="PSUM") as ops_pool,
        ):
            # ---- one-time parameter loads ----
            wk_sb = singles.tile([128, KC, N_PROTO], f16)
            nc.sync.dma_start(
                out=wk_sb, in_=wk_d.rearrange("(a p) q -> p a q", p=128))
            wvb_sb = singles.tile([128, S * D_ATT], f32)
            nc.sync.dma_start(out=wvb_sb, in_=wvb_d)
            id16 = singles.tile([128, 128], f16)
            nc.sync.dma_start(out=id16, in_=id16_d)
            id32 = singles.tile([128, 128], f32)
            nc.sync.dma_start(out=id32, in_=id32_d)
            eps_t = singles.tile([128, 1], f32)
            nc.vector.memset(eps_t, LN_EPS)
            if add_c0:
                c0b = singles.tile([128, N_PROTO], f32)
                nc.sync.dma_start(out=c0b, in_=bcast_load(c0_d, N_PROTO))
            if use_bo:
                bob = singles.tile([128, D_ATT], f32)
                nc.sync.dma_start(out=bob, in_=bcast_load(bo_d, D_ATT))
            if use_gamma:
                gamb = singles.tile([128, D_ATT], f32)
                nc.sync.dma_start(out=gamb, in_=bcast_load(gam_d, D_ATT))
            if use_beta:
                betb = singles.tile([128, D_ATT], f32)
                nc.sync.dma_start(out=betb, in_=bcast_load(bet_d, D_ATT))

            copy_idx = 0
            b0 = 0
            for sc in sizes:
                rows = 128 * sc
                x_c = x_d[b0:b0 + rows].rearrange("(p s) k -> p s k", s=sc)
                mp_c = mp_d[b0:b0 + rows].rearrange("(p s) -> p s", s=sc)
                w_c = w_d[b0:b0 + rows].rearrange("(p s) q -> p s q", s=sc)
                out_c = out_d[b0:b0 + rows].rearrange("(p s) d -> p s d", s=sc)
                b0 += rows
                # ---- focal modulation factor g = SCALE*(1+a*t*u^2) ----
                mp_t = small_pool.tile([128, sc], f32, tag="mp")
                nc.sync.dma_start(out=mp_t, in_=mp_c)
                t_t = small_pool.tile([128, sc], f32, tag="gt")
                nc.vector.tensor_scalar(out=t_t, in0=mp_t, scalar1=2.0,
                                        scalar2=-1.0, op0=ALU.mult,
                                        op1=ALU.add)
                tn_t = small_pool.tile([128, sc], f32, tag="gtn")
                nc.vector.tensor_scalar_mul(out=tn_t, in0=t_t, scalar1=-1.0)
                a_t = small_pool.tile([128, sc], f32, tag="ga")
                nc.vector.tensor_max(out=a_t, in0=t_t, in1=tn_t)
                u_t = small_pool.tile([128, sc], f32, tag="gu")
                nc.vector.tensor_scalar(out=u_t, in0=a_t, scalar1=-1.0,
                                        scalar2=1.0 + 1e-8, op0=ALU.mult,
                                        op1=ALU.add)
                u2_t = small_pool.tile([128, sc], f32, tag="gu2")
                nc.vector.tensor_mul(out=u2_t, in0=u_t, in1=u_t)
                g_t = small_pool.tile([128, sc], f32, tag="gg")
                nc.vector.tensor_scalar(out=g_t, in0=u2_t, scalar1=g_a,
                                        scalar2=g_c, op0=ALU.mult, op1=ALU.add)

                # ---- x load (8-slot DMAs) + PE transposes + scores ----
                scores_ps = scps_pool.tile([128, sc, N_PROTO], f32)
                x_ts = []
                for lg in range(0, sc, 4):
                    x_t = xin_pool.tile([128, 4, D_IN], f16, tag="x")
                    nc.gpsimd.dma_start(out=x_t, in_=x_c[:, lg:lg + 4, :])
                    x_ts.append((lg, 4, x_t))
                for lg, w8, x_t in x_ts:
                    for sg4 in range(0, w8, 4):
                        xt_sbs = []
                        for kc in range(KC):
                            xt_ps = xtps_pool.tile([128, 4, 128], f32)
                            for jj in range(4):
                                # xT block = x_block.T @ I  (f16 matmul)
                                nc.tensor.matmul(
                                    out=xt_ps[:, jj, :],
                                    lhsT=x_t[:, sg4 + jj, 128 * kc:128 * (kc + 1)],
                                    rhs=id16,
                                    start=True, stop=True)
                            xt_sb = xtsb_pool.tile([128, 4, 128], f16, tag="xt")
                            if (copy_idx % copy_mod) < copy_split:
                                nc.scalar.copy(out=xt_sb[:], in_=xt_ps[:])
                            else:
                                nc.vector.tensor_copy(out=xt_sb[:], in_=xt_ps[:])
                            copy_idx += 1
                            xt_sbs.append(xt_sb)
                        for jj in range(4):
                            j = lg + sg4 + jj
                            for kc in range(KC):
                                nc.tensor.matmul(
                                    out=scores_ps[:, j, :],
                                    lhsT=xt_sbs[kc][:, jj, :],
                                    rhs=wk_sb[:, kc, :],
                                    start=(kc == 0), stop=(kc == KC - 1))

                # ---- softmax over prototypes (natural layout) ----
                l_t = smx_pool.tile([128, sc, N_PROTO], f32, tag="logit")
                if add_c0:
                    nc.vector.tensor_add(out=l_t, in0=scores_ps,
                                         in1=_expand(c0b[:], 0, sc, bass))
                    nc.vector.tensor_mul(out=l_t, in0=l_t,
                                         in1=_expand(g_t[:], 1, N_PROTO, bass))
                else:
                    nc.vector.tensor_mul(out=l_t, in0=scores_ps,
                                         in1=_expand(g_t[:], 1, N_PROTO, bass))
                e_t = smx_pool.tile([128, sc, N_PROTO], f32, tag="esc")
                nc.scalar.activation(out=e_t, in_=l_t, func=AF.Exp)
                z_t = small_pool.tile([128, sc], f32, tag="zsum")
                nc.vector.tensor_reduce(out=z_t, in_=e_t, axis=AX.X, op=ALU.add)
                rz_t = small_pool.tile([128, sc], f32, tag="rz")
                nc.vector.reciprocal(out=rz_t, in_=z_t)
                w_t = wout_pool.tile([128, sc, N_PROTO], f32, tag="w")
                nc.vector.tensor_mul(out=w_t, in0=e_t,
                                     in1=_expand(rz_t[:], 1, N_PROTO, bass))
                nc.sync.dma_start(out=w_c, in_=w_t)

                # ---- weights.T then block-diag Wv' matmul -> pre-LN ----
                kq = N_PROTO * sc
                wt_ps = wtps_pool.tile([128, 128], f32)
                w_flat = w_t[:].rearrange("p s q -> p (s q)")
                nc.tensor.matmul(out=wt_ps[:kq, :], lhsT=w_flat, rhs=id32,
                                 start=True, stop=True)
                wt_sb = smx_pool.tile([128, 128], f32, tag="wt")
                nc.scalar.copy(out=wt_sb[:kq, :], in_=wt_ps[:kq, :])


                outp = ops_pool.tile([128, sc, D_ATT], f32)
                for h0 in range(0, sc * D_ATT, 512):
                    hw = min(512, sc * D_ATT - h0)
                    nc.tensor.matmul(
                        out=outp[:, h0 // D_ATT:(h0 + hw) // D_ATT, :],
                        lhsT=wt_sb[:kq, :],
                        rhs=wvb_sb[:kq, h0:h0 + hw],
                        start=True, stop=True)
                ln_src = outp[:]
                if use_bo:
                    y_t = ln_pool.tile([128, sc, D_ATT], f32, tag="y")
                    nc.vector.tensor_add(out=y_t, in0=outp,
                                         in1=_expand(bob[:], 0, sc, bass))
                    ln_src = y_t[:]
                s1_t = small_pool.tile([128, sc], f32, tag="lnsum")
                nc.vector.tensor_reduce(out=s1_t, in_=ln_src, axis=AX.X,
                                        op=ALU.add)
                m_t = small_pool.tile([128, sc], f32, tag="lnmean")
                nc.vector.tensor_scalar_mul(out=m_t, in0=s1_t,
                                            scalar1=1.0 / D_ATT)
                c_t = ln_pool.tile([128, sc, D_ATT], f32, tag="lncen")
                nc.vector.tensor_sub(out=c_t, in0=ln_src,
                                     in1=_expand(m_t[:], 1, D_ATT, bass))
                q_t = ln_pool.tile([128, sc, D_ATT], f32, tag="lnsq")
                nc.scalar.square(out=q_t[:], in_=c_t[:])
                v_t = small_pool.tile([128, sc], f32, tag="lnvar")
                nc.vector.tensor_reduce(out=v_t, in_=q_t, axis=AX.X,
                                        op=ALU.add)
                lnv_t = small_pool.tile([128, sc], f32, tag="lnlog")
                nc.scalar.activation(out=lnv_t, in_=v_t, func=AF.Ln,
                                     scale=1.0 / D_ATT, bias=eps_t[:])
                r_t = small_pool.tile([128, sc], f32, tag="lnrstd")
                nc.scalar.activation(out=r_t, in_=lnv_t, func=AF.Exp,
                                     scale=-0.5)
                o_t = out_pool.tile([128, sc, D_ATT], f32, tag="o")
                nc.vector.tensor_mul(out=o_t, in0=c_t,
                                     in1=_expand(r_t[:], 1, D_ATT, bass))
                if use_gamma:
                    nc.vector.tensor_mul(out=o_t, in0=o_t,
                                         in1=_expand(gamb[:], 0, sc, bass))
                if use_beta:
                    nc.vector.tensor_add(out=o_t, in0=o_t,
                                         in1=_expand(betb[:], 0, sc, bass))
                nc.sync.dma_start(out=out_c, in_=o_t)

    nc.compile()
    return nc


def _prepare(inputs):
    """Fold parameters on the host, build (or reuse) the Bass program and
    the 8 per-core input maps."""
    x = np.ascontiguousarray(np.asarray(inputs["x"], dtype=np.float32))
    mp = np.ascontiguousarray(np.asarray(inputs["minority_prob"], np.float32))
    Wq = np.asarray(inputs["Wq"], np.float32)
    bq = np.asarray(inputs["bq"], np.float32)
    pk = np.asarray(inputs["proto_keys"], np.float32)
    pv = np.asarray(inputs["proto_values"], np.float32)
    imp = np.asarray(inputs["proto_importance"], np.float32)
    alpha = float(np.asarray(inputs["focal_alpha"], np.float32).reshape(-1)[0])
    temp = float(np.asarray(inputs["focal_temp"], np.float32).reshape(-1)[0])
    Wo = np.asarray(inputs["Wo"], np.float32)
    bo = np.asarray(inputs["bo"], np.float32)
    gam = np.asarray(inputs["ln_gamma"], np.float32)
    bet = np.asarray(inputs["ln_beta"], np.float32)

    b_total = x.shape[0]
    assert b_total % N_CORES == 0
    b_core = b_total // N_CORES

    Wk = pk @ Wq                                     # [4, 512]
    wk16 = np.ascontiguousarray(Wk.T).astype(np.float16)
    c0 = bq @ pk.T + imp                             # [4]
    add_c0 = not bool(np.all(np.abs(c0 - c0[0]) < 1e-30))
    Wv = pv @ Wo.T                                   # [4, 32]
    wvb = np.zeros((128, S * D_ATT), np.float32)
    for j in range(S):
        wvb[N_PROTO * j:N_PROTO * (j + 1), D_ATT * j:D_ATT * (j + 1)] = Wv
    g_a = SCALE * alpha * temp
    g_c = SCALE
    use_bo = bool(np.any(bo != 0.0))
    use_gamma = bool(np.any(gam != 1.0))
    use_beta = bool(np.any(bet != 0.0))

    key = (b_core, round(g_a, 12), round(g_c, 12), add_c0, use_bo,
           use_gamma, use_beta)
    if key not in _NC_CACHE:
        _NC_CACHE[key] = _build_nc(b_core, g_a, g_c, add_c0, use_bo,
                                   use_gamma, use_beta)
    nc = _NC_CACHE[key]

    id16 = np.eye(128, dtype=np.float16)
    id32 = np.eye(128, dtype=np.float32)
    in_maps = []
    for i in range(N_CORES):
        m = {
            "x": x[i * b_core:(i + 1) * b_core],
            "mp": mp[i * b_core:(i + 1) * b_core],
            "wk": wk16,
            "wvb": wvb,
            "id16": id16,
            "id32": id32,
        }
        if add_c0:
            m["c0"] = np.ascontiguousarray(c0)
        if use_bo:
            m["bo"] = np.ascontiguousarray(bo)
        if use_gamma:
            m["gam"] = np.ascontiguousarray(gam)
        if use_beta:
            m["bet"] = np.ascontiguousarray(bet)
        in_maps.append(m)
    return nc, in_maps


def _run(inputs, trace=False, **kwargs):
    _import_bass()
    from concourse.bass_utils import run_bass_kernel_spmd
    nc, in_maps = _prepare(inputs)
    res = run_bass_kernel_spmd(nc, in_maps, core_ids=list(range(N_CORES)),
                               trace=trace, **kwargs)
    out = np.concatenate([r["out"] for r in res.results], axis=0)
    wout = np.concatenate([r["wout"] for r in res.results], axis=0)
    return (out, wout), res


def kernel(**inputs):
    (out, wout), _ = _run(inputs, trace=False)
    return out, wout
